# revision 39
# baseline (speedup 1.0000x reference)
"""DirectNormLoss kernel for Trainium2 (Bass/Tile), 8-core data-parallel.

loss = (1/B) * sum_b [ 1 - <s_b, c_{l_b}> / (||c_{l_b}|| * max(||s_b||, ||t_b||)) ]

Sharding: batch split 8 ways (2048 samples/core), T_EMB replicated in DRAM
(rows fetched on demand via indirect-DMA gather). Each core emits a partial
loss scalar; the host sums the 8 partials (the "all-reduce" of the scalar).

Inputs are downcast to bf16 on the host (halves HBM traffic; perturbs the
final 16K-sample averaged loss by only ~3e-7 relative, measured).

Per-core structure (16 tiles of 128 samples x 2048 features):
  - preamble: per-class 1/||c|| computed once from the bf16 table (8
    square+accum passes on DVE), bounced through internal DRAM, then one
    batched indirect gather -> per-sample invc [128, 16]
  - s/t row-blocks arrive as host-packed bf16 chunks, one 2 MiB DMA per
    chunk alternating between the two HWDGE rings
  - center rows gathered from DRAM T_EMB by label via gpsimd indirect DMA
  - ACT: Square+accum row norms; DVE: fused scalar_tensor_tensor dots
  - tiny per-sample chain: rs = invc / sqrt(max(s2, t2)); PE ones-matmul
    partition-reduce; ACT affine -> (B_CORE - total)/B
"""

import numpy as np

import concourse.bass as bass
import concourse.tile as tile
from concourse import bacc, mybir
from concourse.bass_utils import run_bass_kernel_spmd

# Problem constants (hardcoded per contract).
B_FULL = 16384
D = 2048
NUM_CLASS = 1000
N_CORES = 8
B_CORE = B_FULL // N_CORES          # 2048
P = 128                             # SBUF partitions
N_TILES = B_CORE // P               # 16
CHUNK = 2                           # s/t row-block tiles per DMA (2 MiB)
N_CBLK = 8                          # class-table row blocks (8*128 >= 1000)
ND_WEIGHT = 1.0
# tiles whose t2 square runs on DVE instead of ACT (engine load balance)
DVE_T2_TILES = frozenset((7, 8, 9))

_PROG = None


def _build_program():
    nc = bacc.Bacc("TRN2", target_bir_lowering=False, debug=False,
                   num_devices=N_CORES)

    FT = mybir.dt.float32
    BF = mybir.dt.bfloat16
    Alu = mybir.AluOpType
    Act = mybir.ActivationFunctionType

    n_chunks = N_TILES // CHUNK
    # host-packed bf16 [c, p, x, j, d]: x=0 is s, x=1 is t; contiguous per
    # partition so each chunk is one clean 2 MiB DMA.
    st_ap = nc.dram_tensor("st_emb", [n_chunks, P, 2, CHUNK, D],
                           BF, kind="ExternalInput").ap()
    T_ap = nc.dram_tensor("T_EMB", [NUM_CLASS, D], BF,
                          kind="ExternalInput").ap()
    lab_ap = nc.dram_tensor("labels", [B_CORE], mybir.dt.int32,
                            kind="ExternalInput").ap()
    # labels2[c] = (c % 128) * N_CBLK + c // 128 — index into the
    # partition-major invc bounce layout; host-precomputed.
    lab2_ap = nc.dram_tensor("labels2", [B_CORE], mybir.dt.int32,
                             kind="ExternalInput").ap()
    out_ap = nc.dram_tensor("out", [1, 1], mybir.dt.float32,
                            kind="ExternalOutput").ap()

    # labels host-pretransposed: dram[p*N_TILES + t] = labels[t*P + p]
    lab_r = lab_ap.rearrange("(p t) -> p t", t=N_TILES)
    lab2_r = lab2_ap.rearrange("(p t) -> p t", t=N_TILES)

    with tile.TileContext(nc) as tc:
        with (
            tc.tile_pool(name="stio", bufs=3) as stio,
            tc.tile_pool(name="gio", bufs=6) as gio,
            tc.tile_pool(name="tblk", bufs=3) as tblk,
            tc.tile_pool(name="dump", bufs=4) as dump,
            tc.tile_pool(name="stats", bufs=8) as stats,
            tc.tile_pool(name="persist", bufs=1) as persist,
            tc.tile_pool(name="dram", bufs=1, space="DRAM") as drampool,
            tc.tile_pool(name="psum", bufs=1, space="PSUM") as psum_pool,
        ):
            labels_sb = persist.tile([P, N_TILES], mybir.dt.int32)
            nc.sync.dma_start(out=labels_sb[:], in_=lab_r)
            labels2_sb = persist.tile([P, N_TILES], mybir.dt.int32)
            nc.sync.dma_start(out=labels2_sb[:], in_=lab2_r)

            # ---- per-class inverse norms --------------------------------
            c2 = persist.tile([P, N_CBLK], FT)
            nc.vector.memset(c2[:], 1.0)  # pad rows (classes >= 1000)
            for i in range(N_CBLK):
                lo = i * P
                rows = min(P, NUM_CLASS - lo)
                tt = tblk.tile([P, D], BF, tag="tblk")
                nc.sync.dma_start(out=tt[:rows, :], in_=T_ap[lo:lo + rows, :])
                dc = dump.tile([P, D], BF, tag="dump")
                nc.vector.scalar_tensor_tensor(
                    out=dc[:rows, :], in0=tt[:rows, :], scalar=1.0,
                    in1=tt[:rows, :], op0=Alu.mult, op1=Alu.mult,
                    accum_out=c2[:rows, i:i + 1])
            cnorm = persist.tile([P, N_CBLK], FT)
            nc.scalar.activation(out=cnorm[:], in_=c2[:], func=Act.Sqrt)
            invc = persist.tile([P, N_CBLK], FT)
            nc.vector.reciprocal(out=invc[:], in_=cnorm[:])

            # bounce through DRAM (partition-major: dram[p*N_CBLK + i]),
            # then one batched gather -> per-sample 1/||c|| for all tiles.
            invc_d = drampool.tile([P * N_CBLK, 1], FT)
            invc_d_r = invc_d[:].rearrange("(p i) x -> p (i x)", p=P)
            nc.sync.dma_start(out=invc_d_r, in_=invc[:])
            invc_all = persist.tile([P, N_TILES], FT)
            nc.gpsimd.indirect_dma_start(
                out=invc_all[:], out_offset=None, in_=invc_d[:],
                in_offset=bass.IndirectOffsetOnAxis(ap=labels2_sb[:], axis=0),
            )

            # ---- main loop ----------------------------------------------
            acc = persist.tile([P, N_TILES], FT)

            st_chunk = None
            for t in range(N_TILES):
                c, j = divmod(t, CHUNK)
                if j == 0:
                    # One 2 MiB DMA per chunk, alternating HWDGE rings.
                    st_chunk = stio.tile([P, 2, CHUNK, D], BF, tag="st")
                    eng = nc.sync if c % 2 == 0 else nc.scalar
                    eng.dma_start(out=st_chunk[:], in_=st_ap[c])
                s_v = st_chunk[:, 0, j, :]
                t_v = st_chunk[:, 1, j, :]

                g = gio.tile([P, D], BF, tag="g")
                nc.gpsimd.indirect_dma_start(
                    out=g[:], out_offset=None, in_=T_ap[:],
                    in_offset=bass.IndirectOffsetOnAxis(
                        ap=labels_sb[:, t:t + 1], axis=0),
                )

                # Row sums of squares (engine-balanced).
                s2 = stats.tile([P, 1], FT, tag="s2")
                d0 = dump.tile([P, D], BF, tag="dump")
                nc.scalar.activation(out=d0[:], in_=s_v, func=Act.Square,
                                     accum_out=s2[:])
                t2 = stats.tile([P, 1], FT, tag="t2")
                d1 = dump.tile([P, D], BF, tag="dump")
                if t in DVE_T2_TILES:
                    nc.vector.scalar_tensor_tensor(
                        out=d1[:], in0=t_v, scalar=1.0, in1=t_v,
                        op0=Alu.mult, op1=Alu.mult, accum_out=t2[:])
                else:
                    nc.scalar.activation(out=d1[:], in_=t_v, func=Act.Square,
                                         accum_out=t2[:])

                # rs = invc[labels] / sqrt(max(s2, t2))
                m2 = stats.tile([P, 1], FT, tag="m2")
                nc.vector.tensor_tensor(out=m2[:], in0=s2[:], in1=t2[:],
                                        op=Alu.max)
                rnorm = stats.tile([P, 1], FT, tag="rnorm")
                nc.scalar.activation(out=rnorm[:], in_=m2[:], func=Act.Sqrt)
                rmax = stats.tile([P, 1], FT, tag="rmax")
                nc.vector.reciprocal(out=rmax[:], in_=rnorm[:])
                rs = stats.tile([P, 1], FT, tag="rs")
                nc.vector.tensor_tensor(out=rs[:], in0=rmax[:],
                                        in1=invc_all[:, t:t + 1],
                                        op=Alu.mult)

                # acc[:, t] = sum_f (s * rs) * g  (per-sample scaled dot)
                d3 = dump.tile([P, D], BF, tag="dump")
                nc.vector.scalar_tensor_tensor(
                    out=d3[:], in0=s_v, scalar=rs[:], in1=g[:],
                    op0=Alu.mult, op1=Alu.mult,
                    accum_out=acc[:, t:t + 1],
                )

            # partial = (B_CORE - sum(acc)) * ND_WEIGHT / B_FULL
            rsum = persist.tile([P, 1], FT)
            nc.vector.tensor_reduce(out=rsum[:], in_=acc[:],
                                    axis=mybir.AxisListType.X, op=Alu.add)
            ones = persist.tile([P, 1], FT)
            nc.vector.memset(ones[:], 1.0)
            total = psum_pool.tile([1, 1], FT)
            nc.tensor.matmul(out=total[:], lhsT=rsum[:], rhs=ones[:],
                             start=True, stop=True)
            res = persist.tile([1, 1], FT)
            nc.scalar.activation(out=res[:], in_=total[:], func=Act.Copy,
                                 bias=float(B_CORE) * ND_WEIGHT / B_FULL,
                                 scale=-ND_WEIGHT / B_FULL)
            nc.sync.dma_start(out=out_ap[:], in_=res[:])

    nc.compile()
    return nc


def _get_program():
    global _PROG
    if _PROG is None:
        _PROG = _build_program()
    return _PROG


def _pack_st(s_core, t_core):
    """[B_CORE, D] x2 -> bf16 [n_chunks, P, 2, CHUNK, D] in DMA order."""
    import ml_dtypes
    n_chunks = N_TILES // CHUNK
    s4 = s_core.reshape(n_chunks, CHUNK, P, D)
    t4 = t_core.reshape(n_chunks, CHUNK, P, D)
    st = np.stack([s4, t4], axis=2)          # [c, j, x, p, d]
    return np.ascontiguousarray(
        st.transpose(0, 3, 2, 1, 4).astype(ml_dtypes.bfloat16))


def _pretranspose(lab_core):
    """[B_CORE] -> dram[p*N_TILES + t] = lab[t*P + p] (contiguous load)."""
    return np.ascontiguousarray(
        lab_core.reshape(N_TILES, P).T).reshape(B_CORE)


def _make_in_maps(s_emb, t_emb, T_EMB, labels):
    import ml_dtypes
    s_emb = np.asarray(s_emb, dtype=np.float32)
    t_emb = np.asarray(t_emb, dtype=np.float32)
    T_EMB = np.ascontiguousarray(
        np.asarray(T_EMB, dtype=np.float32).astype(ml_dtypes.bfloat16))
    labels_i32 = np.ascontiguousarray(labels.astype(np.int32))
    in_maps = []
    for i in range(N_CORES):
        lo, hi = i * B_CORE, (i + 1) * B_CORE
        lab_core = labels_i32[lo:hi]
        lab2_core = (lab_core % P) * N_CBLK + lab_core // P
        in_maps.append({
            "st_emb": _pack_st(s_emb[lo:hi], t_emb[lo:hi]),
            "T_EMB": T_EMB,
            "labels": _pretranspose(lab_core),
            "labels2": _pretranspose(lab2_core.astype(np.int32)),
        })
    return in_maps


def run(s_emb, t_emb, T_EMB, labels, trace=False, **spmd_kwargs):
    """Run on 8 NeuronCores; returns (loss_scalar, BassKernelResults)."""
    nc = _get_program()
    in_maps = _make_in_maps(s_emb, t_emb, T_EMB, labels)
    res = run_bass_kernel_spmd(nc, in_maps, core_ids=list(range(N_CORES)),
                               trace=trace, **spmd_kwargs)
    partials = [res.results[i]["out"][0, 0] for i in range(N_CORES)]
    loss = np.array(np.sum(np.asarray(partials, dtype=np.float64)),
                    dtype=np.float32)
    return loss, res


def kernel(s_emb, t_emb, T_EMB, labels):
    loss, _ = run(s_emb, t_emb, T_EMB, labels)
    return loss


# revision 41
# speedup vs baseline: 1.0784x; 1.0784x over previous
"""DirectNormLoss kernel for Trainium2 (Bass/Tile), 8-core data-parallel.

loss = (1/B) * sum_b [ 1 - <s_b, c_{l_b}> / (||c_{l_b}|| * max(||s_b||, ||t_b||)) ]

Sharding: batch split 8 ways (2048 samples/core), T_EMB replicated in DRAM
(rows fetched on demand via indirect-DMA gather). Each core emits a partial
loss scalar; the host sums the 8 partials (the "all-reduce" of the scalar).

Inputs are downcast to bf16 on the host (halves HBM traffic; perturbs the
final 16K-sample averaged loss by only ~3e-7 relative, measured).

Per-core structure (16 tiles of 128 samples x 2048 features):
  - s rows arrive sample-major; t rows arrive FEATURE-major, host-packed
    into one bf16 buffer so each 2-tile chunk is a single 2 MiB DMA that is
    contiguous per partition.
  - t row norms come from the otherwise-idle TensorEngine: per tile, 16
    accumulating 128x128 matmuls build the Gram matrix t_T^T @ t_T in PSUM;
    a single fused DVE op (Gram * Identity, row-accumulate) extracts the
    diagonal = per-sample ||t||^2.
  - s norms on ACT (Square + accum_out); center-row norms g2 split between
    ACT and DVE for balance; dots via fused DVE scalar_tensor_tensor.
  - per-sample: rs = 1/sqrt(max(s2,t2) * g2); PE ones-matmul partition
    reduce; ACT affine -> (B_CORE - total)/B.
"""

import numpy as np

import concourse.bass as bass
import concourse.tile as tile
from concourse import bacc, mybir
from concourse.bass_utils import run_bass_kernel_spmd
from concourse.masks import make_identity

# Problem constants (hardcoded per contract).
B_FULL = 16384
D = 2048
NUM_CLASS = 1000
N_CORES = 8
B_CORE = B_FULL // N_CORES          # 2048
P = 128                             # SBUF partitions
N_TILES = B_CORE // P               # 16
CHUNK = 2                           # tiles per DMA chunk
N_FB = D // P                       # 16 feature blocks per tile
ND_WEIGHT = 1.0
# tiles whose g2 square runs on DVE instead of ACT (engine load balance)
DVE_G2_TILES = frozenset((5, 6, 7, 8, 9, 10))

_PROG = None


def _build_program():
    nc = bacc.Bacc("TRN2", target_bir_lowering=False, debug=False,
                   num_devices=N_CORES)

    FT = mybir.dt.float32
    BF = mybir.dt.bfloat16
    Alu = mybir.AluOpType
    Act = mybir.ActivationFunctionType

    n_chunks = N_TILES // CHUNK
    # host-packed bf16, per partition: [s(j,d) | tT(fb,j,srow)] = 8192 elems
    st_ap = nc.dram_tensor("st_emb", [n_chunks, P, 2 * CHUNK * D],
                           BF, kind="ExternalInput").ap()
    T_ap = nc.dram_tensor("T_EMB", [NUM_CLASS, D], BF,
                          kind="ExternalInput").ap()
    lab_ap = nc.dram_tensor("labels", [B_CORE], mybir.dt.int32,
                            kind="ExternalInput").ap()
    out_ap = nc.dram_tensor("out", [1, 1], mybir.dt.float32,
                            kind="ExternalOutput").ap()

    # labels host-pretransposed: dram[p*N_TILES + t] = labels[t*P + p]
    lab_r = lab_ap.rearrange("(p t) -> p t", t=N_TILES)

    with tile.TileContext(nc) as tc:
        with (
            tc.tile_pool(name="stio", bufs=3) as stio,
            tc.tile_pool(name="gio", bufs=6) as gio,
            tc.tile_pool(name="dump", bufs=4) as dump,
            tc.tile_pool(name="stats", bufs=8) as stats,
            tc.tile_pool(name="persist", bufs=1) as persist,
            tc.tile_pool(name="psum", bufs=2, space="PSUM") as psum_pool,
        ):
            labels_sb = persist.tile([P, N_TILES], mybir.dt.int32)
            nc.sync.dma_start(out=labels_sb[:], in_=lab_r)

            ident = persist.tile([P, P], FT)
            make_identity(nc, ident[:])

            acc = persist.tile([P, N_TILES], FT)

            st_chunk = None
            for t in range(N_TILES):
                c, j = divmod(t, CHUNK)
                if j == 0:
                    # One 2 MiB DMA per chunk, alternating HWDGE rings.
                    st_chunk = stio.tile([P, 2 * CHUNK * D], BF, tag="st")
                    eng = nc.sync if c % 2 == 0 else nc.scalar
                    eng.dma_start(out=st_chunk[:], in_=st_ap[c])
                s_v = st_chunk[:, j * D:(j + 1) * D]

                def t_fb(fb, _j=j):
                    off = CHUNK * D + fb * (CHUNK * P) + _j * P
                    return st_chunk[:, off:off + P]

                g = gio.tile([P, D], BF, tag="g")
                nc.gpsimd.indirect_dma_start(
                    out=g[:], out_offset=None, in_=T_ap[:],
                    in_offset=bass.IndirectOffsetOnAxis(
                        ap=labels_sb[:, t:t + 1], axis=0),
                )

                # ||t||^2 via TensorEngine Gram accumulation + diag extract.
                gram = psum_pool.tile([P, P], FT, tag="gram")
                for fb in range(N_FB):
                    nc.tensor.matmul(out=gram[:], lhsT=t_fb(fb),
                                     rhs=t_fb(fb),
                                     start=(fb == 0), stop=(fb == N_FB - 1))
                t2 = stats.tile([P, 1], FT, tag="t2")
                dg = dump.tile([P, P], FT, tag="diag")
                nc.vector.scalar_tensor_tensor(
                    out=dg[:], in0=gram[:], scalar=1.0, in1=ident[:],
                    op0=Alu.mult, op1=Alu.mult, accum_out=t2[:])

                # ||s||^2 on ACT; ||g||^2 split ACT/DVE for balance.
                s2 = stats.tile([P, 1], FT, tag="s2")
                d0 = dump.tile([P, D], BF, tag="dump")
                nc.scalar.activation(out=d0[:], in_=s_v, func=Act.Square,
                                     accum_out=s2[:])
                g2 = stats.tile([P, 1], FT, tag="g2")
                d2 = dump.tile([P, D], BF, tag="dump")
                if t in DVE_G2_TILES:
                    nc.vector.scalar_tensor_tensor(
                        out=d2[:], in0=g[:], scalar=1.0, in1=g[:],
                        op0=Alu.mult, op1=Alu.mult, accum_out=g2[:])
                else:
                    nc.scalar.activation(out=d2[:], in_=g[:],
                                         func=Act.Square, accum_out=g2[:])

                # rs = 1 / sqrt(max(s2, t2) * g2)
                m2 = stats.tile([P, 1], FT, tag="m2")
                nc.vector.tensor_tensor(out=m2[:], in0=s2[:], in1=t2[:],
                                        op=Alu.max)
                p2 = stats.tile([P, 1], FT, tag="p2")
                nc.vector.tensor_tensor(out=p2[:], in0=m2[:], in1=g2[:],
                                        op=Alu.mult)
                rnorm = stats.tile([P, 1], FT, tag="rnorm")
                nc.scalar.activation(out=rnorm[:], in_=p2[:], func=Act.Sqrt)
                rs = stats.tile([P, 1], FT, tag="rs")
                nc.vector.reciprocal(out=rs[:], in_=rnorm[:])

                # acc[:, t] = sum_f (s * rs) * g  (per-sample scaled dot)
                d3 = dump.tile([P, D], BF, tag="dump")
                nc.vector.scalar_tensor_tensor(
                    out=d3[:], in0=s_v, scalar=rs[:], in1=g[:],
                    op0=Alu.mult, op1=Alu.mult,
                    accum_out=acc[:, t:t + 1],
                )

            # partial = (B_CORE - sum(acc)) * ND_WEIGHT / B_FULL
            rsum = persist.tile([P, 1], FT)
            nc.vector.tensor_reduce(out=rsum[:], in_=acc[:],
                                    axis=mybir.AxisListType.X, op=Alu.add)
            ones = persist.tile([P, 1], FT)
            nc.vector.memset(ones[:], 1.0)
            total = psum_pool.tile([1, 1], FT, tag="total")
            nc.tensor.matmul(out=total[:], lhsT=rsum[:], rhs=ones[:],
                             start=True, stop=True)
            res = persist.tile([1, 1], FT)
            nc.scalar.activation(out=res[:], in_=total[:], func=Act.Copy,
                                 bias=float(B_CORE) * ND_WEIGHT / B_FULL,
                                 scale=-ND_WEIGHT / B_FULL)
            nc.sync.dma_start(out=out_ap[:], in_=res[:])

    nc.compile()
    return nc


def _get_program():
    global _PROG
    if _PROG is None:
        _PROG = _build_program()
    return _PROG


def _pack_st(s_core, t_core):
    """-> bf16 [n_chunks, P, 2*CHUNK*D]: s sample-major, t feature-major."""
    import ml_dtypes
    n_chunks = N_TILES // CHUNK
    s_part = (s_core.reshape(n_chunks, CHUNK, P, D)
              .transpose(0, 2, 1, 3)           # [c, p, j, d]
              .reshape(n_chunks, P, CHUNK * D))
    t_part = (t_core.reshape(n_chunks, CHUNK, P, N_FB, P)  # [c,j,srow,fb,pf]
              .transpose(0, 4, 3, 1, 2)        # [c, pf, fb, j, srow]
              .reshape(n_chunks, P, CHUNK * D))
    st = np.concatenate([s_part, t_part], axis=2)
    return np.ascontiguousarray(st.astype(ml_dtypes.bfloat16))


def _pretranspose(lab_core):
    """[B_CORE] -> dram[p*N_TILES + t] = lab[t*P + p] (contiguous load)."""
    return np.ascontiguousarray(
        lab_core.reshape(N_TILES, P).T).reshape(B_CORE)


def _make_in_maps(s_emb, t_emb, T_EMB, labels):
    import ml_dtypes
    s_emb = np.asarray(s_emb, dtype=np.float32)
    t_emb = np.asarray(t_emb, dtype=np.float32)
    T_EMB = np.ascontiguousarray(
        np.asarray(T_EMB, dtype=np.float32).astype(ml_dtypes.bfloat16))
    labels_i32 = np.ascontiguousarray(labels.astype(np.int32))
    in_maps = []
    for i in range(N_CORES):
        lo, hi = i * B_CORE, (i + 1) * B_CORE
        in_maps.append({
            "st_emb": _pack_st(s_emb[lo:hi], t_emb[lo:hi]),
            "T_EMB": T_EMB,
            "labels": _pretranspose(labels_i32[lo:hi]),
        })
    return in_maps


def run(s_emb, t_emb, T_EMB, labels, trace=False, **spmd_kwargs):
    """Run on 8 NeuronCores; returns (loss_scalar, BassKernelResults)."""
    nc = _get_program()
    in_maps = _make_in_maps(s_emb, t_emb, T_EMB, labels)
    res = run_bass_kernel_spmd(nc, in_maps, core_ids=list(range(N_CORES)),
                               trace=trace, **spmd_kwargs)
    partials = [res.results[i]["out"][0, 0] for i in range(N_CORES)]
    loss = np.array(np.sum(np.asarray(partials, dtype=np.float64)),
                    dtype=np.float32)
    return loss, res


def kernel(s_emb, t_emb, T_EMB, labels):
    loss, _ = run(s_emb, t_emb, T_EMB, labels)
    return loss
